# revision 21
# baseline (speedup 1.0000x reference)
"""Trainium2 Bass kernel for nn_CrossAttention (B=2, N=2048, D=1024, H=16).

Sharding (8 cores): core c -> (batch b = c//4, head-group hg = c%4).
Each head-group is 4 heads = 256 of the 1024 projection dims.

v5 design:
  - All matmul inputs bf16 (host converts); PSUM accumulation fp32.
  - X^T staged fully resident in SBUF; input DMAs upfront, split in two
    per 512-token chunk so the first projection starts early.
  - Scores for the 2 heads of a half-group go into ONE [128,1024] psum
    tile (2 banks) so exp is a single [128,1024] Activation instruction.
  - Inner kt loop software-pipelined: scores(kt+1) issued before PV(kt).
  - V layout per (kv-tile, head-pair): [v_h0 | ones | v_h1] (129 cols).
    j=0 PV lhsT = cols 0..64  -> xa0[0:65]   (x rows 0-63, denom row 64)
    j=1 PV lhsT = cols 1..128 -> xa1[0:128]  (denom row 63, x rows 64-127)
    so BOTH head outputs land partition-aligned for the DVE multiplies
    and no partition-shift DMA is needed.
  - Softmax normalize is engine-only (immune to DMA/collective traffic):
    copy xa->sbuf (frees psum fast), DVE 32x32 stream-transpose reshapes
    the 512 denominators to [32,16], reciprocal there (cheap), transpose
    back, then a K=1 PE matmul broadcasts the bf16 reciprocal row into a
    [128,512] psum tile (both heads at tile_position cols 0/64).  The
    broadcast matmuls + multiplies are deferred into the next block's
    kt loop so the PE never waits on the DVE chain.
  - Out-projection reuses the score psum pool; bias bo is added on host.
  - Per-q-tile bf16 ReduceScatter overlapped with the next tile's
    compute; the last tile is split into 2x256-token chunks to shorten
    the exposed tail.  Tail loads/conversions all sit at program end so
    the in-order SP DMA queue never head-blocks on a collective.
"""

import numpy as np

B = 2
NT = 2048
D = 1024
HEADS = 16
DH = 64
NCORES = 8
CPB = 4  # cores per batch
HG = 4   # heads per core
HGD = HG * DH  # 256 cols per core
GROUPS = [[0, 1, 2, 3], [4, 5, 6, 7]]
SCALE = DH ** -0.5
NQT = 4     # q tiles of 512
NKT = 16    # kv tiles of 128
VB = 2 * DH + 1  # v columns per head-pair block: [v_h0 | ones | v_h1]

_patched = False


def _patch_tile_drain():
    """This container's walrus rejects >1 sync-wait on a Drain
    (CoreV3GenImpl setupSyncWait<CTRL_NO_STRUCT>: "Too many sync wait
    commands").  Split the final TileContext drain's waits across a chain
    of single-wait drains; semaphores are monotonic so sequential waits
    are equivalent to one multi-wait."""
    global _patched
    if _patched:
        return
    import concourse.tile as tile
    import concourse.mybir as mybir
    from concourse.vector_clock import ScopedClock

    _uid = [0]

    def _split_multiwaits(nc):
        for f in nc.m.functions:
            for bb in f.blocks:
                il = bb.instructions
                i = 0
                while i < len(il):
                    inst = il[i]
                    si = inst.sync_info
                    if si is not None and len(si.on_wait) > 1:
                        waits = list(si.on_wait)
                        inst.sync_info = mybir.SyncInfo(
                            on_wait=[waits[-1]], on_update=list(si.on_update)
                        )
                        for w in waits[:-1]:
                            _uid[0] += 1
                            nop = mybir.InstEventSemaphore(
                                name=f"WSPLIT-{_uid[0]}",
                                engine=inst.engine,
                                ins=[],
                                outs=[],
                                sync_info=mybir.SyncInfo(
                                    on_wait=[w], on_update=[]),
                            )
                            il.insert(i, nop)
                            i += 1
                    i += 1

    def _drain_and_barrier(self, tick_clock, wait_clock):
        nc = self.nc
        drain_inst = nc.sync.drain()
        wait_clock.add_sem_waits(
            drain_inst.ins, ScopedClock({None: tick_clock.global_clock})
        )
        si = drain_inst.ins.sync_info
        if si is not None and len(si.on_wait) > 1:
            waits = list(si.on_wait)
            drain_inst.ins.sync_info = mybir.SyncInfo(
                on_wait=[waits[0]], on_update=list(si.on_update)
            )
            for w in waits[1:]:
                extra = nc.sync.drain()
                extra.ins.sync_info = mybir.SyncInfo(on_wait=[w], on_update=[])

        _split_multiwaits(nc)
        nc.all_engine_barrier()
        assert self.sems is not None
        popped = nc._tile_sem_poison_stack.pop()
        assert popped is self._sem_poison
        nc.clear_and_free_semaphores(list(self.sems.allocated().values()))
        nc.all_engine_barrier()

    tile.TileContext._drain_and_barrier = _drain_and_barrier
    _patched = True


def build_program():
    _patch_tile_drain()
    import concourse.bass as bass
    import concourse.tile as tile
    import concourse.mybir as mybir

    f32 = mybir.dt.float32
    bf16 = mybir.dt.bfloat16
    EXP = mybir.ActivationFunctionType.Exp

    nc = bass.Bass("TRN2", target_bir_lowering=False, debug=False,
                   num_devices=NCORES)

    xqT = nc.dram_tensor("xqT", [D, NT], bf16, kind="ExternalInput")
    xkT = nc.dram_tensor("xkT", [D, NT], bf16, kind="ExternalInput")
    xvT = nc.dram_tensor("xvT", [D, NT], bf16, kind="ExternalInput")
    wqT = nc.dram_tensor("wqT", [D, HGD], bf16, kind="ExternalInput")
    wkT = nc.dram_tensor("wkT", [D, HGD], bf16, kind="ExternalInput")
    wvT = nc.dram_tensor("wvT", [D, HGD], bf16, kind="ExternalInput")
    woT = nc.dram_tensor("woT", [HGD, D], bf16, kind="ExternalInput")
    out = nc.dram_tensor("out", [2 * NQT, 64, D], bf16, kind="ExternalOutput")

    partial = nc.dram_tensor("partial", [NT, D], bf16)
    rsout = nc.dram_tensor("rsout", [2 * NQT, 64, D], bf16)

    with tile.TileContext(nc) as tc:
        from contextlib import ExitStack
        with ExitStack() as ctx:
            const = ctx.enter_context(tc.tile_pool(name="const", bufs=1))
            persist = ctx.enter_context(tc.tile_pool(name="persist", bufs=1))
            pt_pool = ctx.enter_context(tc.tile_pool(name="pt", bufs=3))
            outsb = ctx.enter_context(tc.tile_pool(name="outsb", bufs=2))
            obp = ctx.enter_context(tc.tile_pool(name="obp", bufs=8))
            # PSUM (8 banks of [128,2KB]): st = 2x[128,1024]f32 (4 banks,
            # scores + projection/out-proj accumulators), xa0/xa1 = PV
            # accumulators (2 banks), bc = broadcast reciprocals (1 bank).
            st_ps = ctx.enter_context(
                tc.tile_pool(name="st_ps", bufs=2, space="PSUM"))
            xa_ps = ctx.enter_context(
                tc.tile_pool(name="xa_ps", bufs=1, space="PSUM"))

            # --- constants + all input DMAs upfront ------------------------
            wq_sb = const.tile([128, 8, HGD], bf16)  # [k-part, k-tile, col]
            wk_sb = const.tile([128, 8, HGD], bf16)
            wv_sb = const.tile([128, 8, HGD], bf16)
            wo_sb = const.tile([128, 2, D], bf16)    # [d-part, hg k-tile, od]
            xk_st = persist.tile([128, 8, NT], bf16)
            xv_st = persist.tile([128, 8, NT], bf16)
            xq_st = persist.tile([128, 8, NT], bf16)
            # DMA issue order = consumption order; two DMAs per chunk so
            # the first projection matmul isn't gated on one 1MB transfer.
            nc.sync.dma_start(out=wk_sb[:], in_=wkT[:].rearrange(
                "(t p) c -> p t c", p=128))
            for st_, src in ((xk_st, xkT), (xv_st, xvT), (xq_st, xqT)):
                for n in range(4):
                    sl = slice(512 * n, 512 * (n + 1))
                    for h, eng in enumerate((nc.sync, nc.scalar)):
                        eng.dma_start(
                            out=st_[:, 4 * h:4 * (h + 1), sl],
                            in_=src[512 * h:512 * (h + 1), sl]
                            .rearrange("(t p) c -> p t c", p=128))
                if st_ is xk_st:
                    nc.sync.dma_start(out=wv_sb[:], in_=wvT[:].rearrange(
                        "(t p) c -> p t c", p=128))
                elif st_ is xv_st:
                    nc.sync.dma_start(out=wq_sb[:], in_=wqT[:].rearrange(
                        "(t p) c -> p t c", p=128))
            nc.sync.dma_start(out=wo_sb[:], in_=woT[:].rearrange(
                "(t p) c -> p t c", p=128))

            # --- persistent activations -----------------------------------
            qt_sb = persist.tile([128, 2, NT], bf16)  # [qcol%128, qcol//128, tok]
            kt_sb = persist.tile([128, 2, NT], bf16)
            v_sb = persist.tile([128, NKT, 2 * VB], bf16)
            xt_sb = persist.tile([128, 2, NT], bf16)  # normalized x^T

            nc.gpsimd.memset(v_sb[:], 1.0)

            # normalize scratch (persist; memset once so the stream
            # transposes never read uninitialized SBUF)
            xu0 = persist.tile([96, 512], f32)    # x(h even) + denom row 64
            xu1 = persist.tile([128, 512], f32)   # denom row 63, x rows 64+
            tr1a = persist.tile([96, 16, 32], f32)
            tr1b = persist.tile([64, 16, 32], f32)
            trba = persist.tile([96, 16, 32], bf16)
            trbb = persist.tile([64, 16, 32], bf16)
            tr3a = persist.tile([96, 512], bf16)
            tr3b = persist.tile([64, 512], bf16)
            ones_sb = const.tile([128, DH], bf16)
            nc.gpsimd.memset(xu0[:], 1.0)
            nc.gpsimd.memset(xu1[:], 1.0)
            nc.gpsimd.memset(tr1a[:], 1.0)
            nc.gpsimd.memset(tr1b[:], 1.0)
            nc.gpsimd.memset(trba[:], 1.0)
            nc.gpsimd.memset(trbb[:], 1.0)
            nc.gpsimd.memset(ones_sb[:], 1.0)

            # --- Q/K projections ------------------------------------------
            def qk_proj(xst, wsb, dst):
                # n-pairs share each loaded weight tile (halves Ldweights);
                # acc[n] regions m=0/m=1 sit in different banks, and each
                # bank hosts a single sequential accumulation group.
                for np_ in range(2):
                    accs = [st_ps.tile([128, 1024], f32, tag="st",
                                       name="qkacc") for _ in range(2)]
                    for k in range(8):
                        for m in range(2):
                            for i, n in enumerate((2 * np_, 2 * np_ + 1)):
                                nc.tensor.matmul(
                                    accs[i][:, 512 * m:512 * (m + 1)],
                                    wsb[:, k, 128 * m:128 * (m + 1)],
                                    xst[:, k, 512 * n:512 * (n + 1)],
                                    start=(k == 0), stop=(k == 7))
                    for i, n in enumerate((2 * np_, 2 * np_ + 1)):
                        for m in range(2):
                            nc.vector.tensor_copy(
                                dst[:, m, 512 * n:512 * (n + 1)],
                                accs[i][:, 512 * m:512 * (m + 1)])

            qk_proj(xk_st, wk_sb, kt_sb)

            # --- V projection ---------------------------------------------
            for mg in range(4):
                acc = st_ps.tile([128, 1024], f32, tag="st", name="vacc")
                # m outer / k inner: two regions share a psum bank, so each
                # region's accumulation must complete before the next opens.
                for m in range(4):
                    for k in range(8):
                        nc.tensor.matmul(
                            acc[:, 256 * m:256 * m + HGD],
                            xv_st[:, k, 512 * mg + 128 * m:
                                  512 * mg + 128 * (m + 1)],
                            wv_sb[:, k, :],
                            start=(k == 0), stop=(k == 7))
                for m in range(4):
                    mt = 4 * mg + m
                    dstv = v_sb[:, mt, :].rearrange("p (g c) -> p g c", c=VB)
                    srcv = acc[:, 256 * m:256 * (m + 1)].rearrange(
                        "p (g c) -> p g c", c=128)
                    # heads {0,2} -> cols 0..63; heads {1,3} -> cols 65..128
                    nc.vector.tensor_copy(dstv[:, :, 0:DH],
                                          srcv[:, :, 0:DH])
                    nc.vector.tensor_copy(dstv[:, :, DH + 1:VB],
                                          srcv[:, :, DH:2 * DH])

            qk_proj(xq_st, wq_sb, qt_sb)

            # --- attention ------------------------------------------------
            partial_dmas = [[] for _ in range(NQT)]
            rs_ccs = []

            def att_hp(qt, hp, pending=None):
                """Emit one (q-tile, head-pair) attention block.  Returns a
                closure that finishes its normalization (PE broadcast +
                DVE multiplies); the caller threads it into the NEXT
                block's kt loop so the PE never stalls on the DVE chain."""
                qsl = slice(512 * qt, 512 * (qt + 1))
                xa0 = xa_ps.tile([DH + 1, 512], f32, tag="xa0")
                xa1 = xa_ps.tile([128, 512], f32, tag="xa1")

                def scores(kt):
                    st = st_ps.tile([128, 1024], f32, tag="st", name="st")
                    for j in range(2):
                        p0 = 64 * j
                        nc.tensor.matmul(
                            st[:, 512 * j:512 * (j + 1)],
                            kt_sb[p0:p0 + 64, hp,
                                  128 * kt:128 * (kt + 1)],
                            qt_sb[p0:p0 + 64, hp, qsl],
                            tile_position=(p0, 0))
                    return st

                def exp(st):
                    pt = pt_pool.tile([128, 1024], bf16)
                    nc.scalar.activation(pt[:], st[:], EXP, scale=SCALE)
                    return pt

                def pv(kt, pt):
                    nc.tensor.matmul(
                        xa0[:],
                        v_sb[:, kt, VB * hp:VB * hp + DH + 1],
                        pt[:, 0:512],
                        start=(kt == 0), stop=(kt == NKT - 1))
                    nc.tensor.matmul(
                        xa1[:],
                        v_sb[:, kt, VB * hp + 1:VB * hp + 129],
                        pt[:, 512:1024],
                        start=(kt == 0), stop=(kt == NKT - 1))

                pt_prev = exp(scores(0))
                for kt in range(1, NKT):
                    st = scores(kt)
                    if kt == 3 and pending is not None:
                        pending()
                    pv(kt - 1, pt_prev)
                    pt_prev = exp(st)
                pv(NKT - 1, pt_prev)

                # DVE-only denominator chain (no DMA -> immune to RS
                # traffic): free the PV psum, reshape denoms via stream
                # transpose, reciprocal at [32,16], transpose back.
                nc.vector.tensor_copy(xu0[0:DH + 1, :], xa0[:])
                nc.vector.tensor_copy(xu1[32:64, :], xa1[32:64, :])
                nc.vector.tensor_copy(xu1[64:128, :], xa1[64:128, :])
                f1a = tr1a[64:96, :, :].rearrange("p a b -> p (a b)")
                nc.vector.transpose(f1a, xu0[64:96, :])
                f1b = tr1b[32:64, :, :].rearrange("p a b -> p (a b)")
                nc.vector.transpose(f1b, xu1[32:64, :])
                # denom of j0 sits at window row 0 -> strided col 0;
                # denom of j1 sits at window row 31 -> strided col 31.
                nc.vector.reciprocal(tr1a[64:96, :, 0:1],
                                     tr1a[64:96, :, 0:1])
                nc.vector.reciprocal(tr1b[32:64, :, 31:32],
                                     tr1b[32:64, :, 31:32])
                nc.vector.tensor_copy(trba[64:96, :, 0:1],
                                      tr1a[64:96, :, 0:1])
                # write j1 recips at strided col 0 so they transpose back
                # to the 32-aligned row 32.
                nc.vector.tensor_copy(trbb[32:64, :, 0:1],
                                      tr1b[32:64, :, 31:32])
                nc.vector.transpose(
                    tr3a[64:96, :], trba[64:96, :, :]
                    .rearrange("p a b -> p (a b)"))
                nc.vector.transpose(
                    tr3b[32:64, :], trbb[32:64, :, :]
                    .rearrange("p a b -> p (a b)"))

                def finish():
                    bc = xa_ps.tile([128, 512], f32, tag="bc")
                    nc.tensor.matmul(bc[0:DH, :], ones_sb[64:65, :],
                                     tr3a[64:65, :], tile_position=(64, 0))
                    nc.tensor.matmul(bc[DH:128, :], ones_sb[32:33, :],
                                     tr3b[32:33, :], tile_position=(32, 64))
                    nc.vector.tensor_mul(
                        xt_sb[0:DH, hp, qsl], xu0[0:DH, :], bc[0:DH, :])
                    nc.vector.tensor_mul(
                        xt_sb[DH:128, hp, qsl], xu1[DH:128, :],
                        bc[DH:128, :])

                return finish

            def outproj_rs(qt):
                # out-proj: partial[t, o] = sum_d x^T[d, t] Wo^T[d, o]
                # (bias added on host).  qt 0-2: one 512-token RS chunk;
                # qt 3: two 256-token chunks to shorten the exposed tail.
                def op_step(acc, n, m):
                    osl = slice(512 * n, 512 * (n + 1))
                    tsl = slice(512 * qt + 128 * m,
                                512 * qt + 128 * (m + 1))
                    for k in range(2):
                        nc.tensor.matmul(
                            acc[:], xt_sb[:, k, tsl], wo_sb[:, k, osl],
                            start=(k == 0), stop=(k == 1))
                    ob = obp.tile([128, 512], bf16, tag="ob")
                    nc.scalar.copy(ob[:], acc[:])
                    w = nc.sync.dma_start(out=partial[tsl, osl], in_=ob[:])
                    partial_dmas[qt].append(w)

                def emit_rs(tok0, ntok, out_ap):
                    # collectives cannot write IO tensors; bounce through
                    # internal DRAM, copied out by gpsimd SW-DGE at the end
                    cc = nc.gpsimd.collective_compute(
                        "ReduceScatter",
                        mybir.AluOpType.add,
                        replica_groups=GROUPS,
                        ins=[partial[tok0:tok0 + ntok, :]],
                        outs=[out_ap],
                    )
                    for w in partial_dmas[qt]:
                        tile.add_dep_helper(cc.ins, w.ins,
                                            reason="RS after partial")
                    rs_ccs.append(cc)

                # m outer: 256-token RS chunks fire after every pair of
                # token strips, overlapping the remaining out-proj work.
                # The last tile issues ONE 512-token chunk instead: each
                # RS is latency-bound (~12us), so two serial small chunks
                # after the final out-proj would cost more than one.
                steps = [(n, m) for m in range(4) for n in range(2)]
                for i, (n, m) in enumerate(steps):
                    if i % 2 == 0:
                        stt = st_ps.tile([128, 1024], f32, tag="st",
                                         name="opacc")
                    op_step(stt[:, 512 * (i % 2):512 * (i % 2 + 1)], n, m)
                    if i == 3 and qt < NQT - 1:
                        emit_rs(512 * qt, 256, rsout[2 * qt])
                if qt < NQT - 1:
                    emit_rs(512 * qt + 256, 256, rsout[2 * qt + 1])
                else:
                    emit_rs(512 * qt, 512,
                            rsout[2 * qt:2 * qt + 2].rearrange(
                                "a p c -> (a p) c"))

            # schedule: normalize finish of block i lands inside block
            # i+1's kt loop; out-proj/RS of tile qt-1 sits between the two
            # head-pair blocks of tile qt.
            fin = att_hp(0, 0)
            fin = att_hp(0, 1, pending=fin)
            for qt in range(1, NQT):
                fin2 = att_hp(qt, 0, pending=fin)
                outproj_rs(qt - 1)
                fin = att_hp(qt, 1, pending=fin2)
            fin()
            outproj_rs(NQT - 1)

            # final out-copies on the gpsimd queue: it is already
            # serialized with the collective stream, so waiting there is
            # free, and the in-order SP/DVE/ACT queues never see an
            # RS-dependent instruction.
            for ch in range(2 * NQT):
                cp = nc.gpsimd.dma_start(out=out[ch], in_=rsout[ch])
                cc = rs_ccs[min(ch, len(rs_ccs) - 1)]
                tile.add_dep_helper(cp.ins, cc.ins, reason="out after RS")

    return nc


_CACHE = {}


def _get_program():
    if "nc" not in _CACHE:
        _CACHE["nc"] = build_program()
    return _CACHE["nc"]


def _bf16(x):
    import ml_dtypes
    return np.ascontiguousarray(np.asarray(x, dtype=ml_dtypes.bfloat16))


def make_in_maps(query, key, value, Wq, Wk, Wv, Wo, bo):
    """Host-side sharding: per-core input dicts."""
    query = np.asarray(query, dtype=np.float32)
    key = np.asarray(key, dtype=np.float32)
    value = np.asarray(value, dtype=np.float32)
    Wq = np.asarray(Wq, dtype=np.float32)
    Wk = np.asarray(Wk, dtype=np.float32)
    Wv = np.asarray(Wv, dtype=np.float32)
    Wo = np.asarray(Wo, dtype=np.float32)

    xT = [_bf16(x.T) for x in
          (query[0], key[0], value[0], query[1], key[1], value[1])]
    wq_c, wk_c, wv_c, wo_c = [], [], [], []
    for hg in range(CPB):
        hsl = slice(HGD * hg, HGD * (hg + 1))
        wq_c.append(_bf16(Wq[hsl, :].T))
        wk_c.append(_bf16(Wk[hsl, :].T))
        wv_c.append(_bf16(Wv[hsl, :].T))
        wo_c.append(_bf16(Wo[:, hsl].T))
    in_maps = []
    for c in range(NCORES):
        b, hg = divmod(c, CPB)
        in_maps.append({
            "xqT": xT[3 * b + 0],
            "xkT": xT[3 * b + 1],
            "xvT": xT[3 * b + 2],
            "wqT": wq_c[hg],
            "wkT": wk_c[hg],
            "wvT": wv_c[hg],
            "woT": wo_c[hg],
        })
    return in_maps


def assemble(results):
    """8 reduce-scatter chunks of 256 tokens; within chunk c, core i (its
    rank in the 4-core group) owns tokens 256*c + 64*i .. + 64."""
    out = np.empty((B, NT, D), dtype=np.float32)
    for c in range(NCORES):
        b, i = divmod(c, CPB)
        o = np.asarray(results[c]["out"], dtype=np.float32)
        for ch in range(2 * (NQT - 1)):
            s = 256 * ch + 64 * i
            out[b, s:s + 64, :] = o[ch]
        # last tile was one 512-token chunk: core i owns 1536 + 128*i
        s = 512 * (NQT - 1) + 128 * i
        out[b, s:s + 64, :] = o[2 * NQT - 2]
        out[b, s + 64:s + 128, :] = o[2 * NQT - 1]
    return out


def run(query, key, value, Wq, Wk, Wv, Wo, bo, trace=False):
    from concourse.bass_utils import run_bass_kernel_spmd
    nc = _get_program()
    in_maps = make_in_maps(query, key, value, Wq, Wk, Wv, Wo, bo)
    res = run_bass_kernel_spmd(nc, in_maps, core_ids=list(range(NCORES)),
                               trace=trace)
    out = assemble(res.results)
    out += np.asarray(bo, dtype=np.float32)
    return out, res


def kernel(query, key, value, qpos=None, kpos=None, Wq=None, Wk=None,
           Wv=None, Wo=None, bo=None):
    out, _ = run(query, key, value, Wq, Wk, Wv, Wo, bo)
    return out
